# revision 12
# baseline (speedup 1.0000x reference)
"""Trainium2 Bass kernel for nn_DecoderRNN (2-layer GRU decoder step + log_softmax).

Model (per reference):
    x  = relu(emb[input_vector])                    [B, E]
    h0 = gru_cell(x,  hidden[0], w_ih0, w_hh0, b_ih0, b_hh0)
    h1 = gru_cell(h0, hidden[1], w_ih1, w_hh1, b_ih1, b_hh1)
    out = log_softmax(h1 @ w_out.T + b_out)         [B, V]
    returns (out, stack([h0, h1]))

Sharding (8 NeuronCores):
  - The GRU (B=128, H=1024) is replicated on every core: at B=128 the matmul
    time is set by the moving (gate) dimension, so batch/tensor splits of the
    GRU only add collectives, which cost ~12-60us each on this runtime.
  - The output projection (V=32001) is column-parallel: each core owns a
    4096-wide vocab shard of w_out.T/b_out, computes its logits shard and the
    local sum(exp(logits)); one AllGather exchanges the per-core sums so every
    core can normalize its shard (log_softmax) on device.
  - A dependency-free tiny AllGather is issued at kernel start to absorb the
    per-execution collective-firmware warmup (~60us) under the weight DMA.

All matmuls run in bf16 (weights pre-cast on host) with fp32 PSUM
accumulation; gate math, softmax math, and outputs are fp32.
"""

import numpy as np
import ml_dtypes

import concourse.bass as bass
import concourse.mybir as mybir
import concourse.tile as tile
from concourse import bacc
from concourse.bass_utils import run_bass_kernel_spmd
from concourse.masks import make_identity

# Problem constants (hardcoded per harness contract)
B = 128
E = 1024
H = 1024
V = 32001
NCORES = 8
VS = 4096            # vocab shard per core (8 * 4096 = 32768 >= 32001, padded)
VPAD = NCORES * VS
P = 128
KC = E // P          # 8 contraction chunks of 128
NEG_BIG = -1e30      # bias value for padded vocab entries -> exp() == 0

f32 = mybir.dt.float32
bf16 = mybir.dt.bfloat16
i32 = mybir.dt.int32
AF = mybir.ActivationFunctionType
ALU = mybir.AluOpType

_CACHE = {}


def _build():
    nc = bacc.Bacc("TRN2", target_bir_lowering=False, debug=False,
                   num_devices=NCORES)

    # ---- I/O ----
    emb_d = nc.dram_tensor("emb", [V, E], bf16, kind="ExternalInput").ap()
    idx_d = nc.dram_tensor("idx", [B, 1], i32, kind="ExternalInput").ap()
    hid_d = nc.dram_tensor("hid", [2, B, H], f32, kind="ExternalInput").ap()
    hidT_d = nc.dram_tensor("hidT", [2, H, B], bf16, kind="ExternalInput").ap()
    wT_d = nc.dram_tensor("wT", [4, E, 3 * H], bf16, kind="ExternalInput").ap()
    gb_d = nc.dram_tensor("gb", [4, 1, 3 * H], bf16, kind="ExternalInput").ap()
    woT_d = nc.dram_tensor("woT", [H, VS], bf16, kind="ExternalInput").ap()
    bo_d = nc.dram_tensor("bo", [1, VS], bf16, kind="ExternalInput").ap()

    out_d = nc.dram_tensor("out", [B, VS], f32, kind="ExternalOutput").ap()
    hout_d = nc.dram_tensor("hout", [2, B, H], f32, kind="ExternalOutput").ap()

    with tile.TileContext(nc) as tc:
        with (
            tc.tile_pool(name="persist", bufs=1) as persist,
            tc.tile_pool(name="wst", bufs=3) as wst,
            tc.tile_pool(name="tmp", bufs=3) as tmp,
            tc.tile_pool(name="dram", bufs=1, space="DRAM") as dram,
            tc.tile_pool(name="ps_tr", bufs=2, space="PSUM") as ps_tr,
            tc.tile_pool(name="ps_g", bufs=1, space="PSUM") as ps_g,
        ):
            # --- dummy collective: starts the ncfw warmup clock at t~0 ---
            warm_in = dram.tile([1, 16], f32, tag="warm_in")
            warm_out = dram.tile([NCORES, 16], f32, tag="warm_out")
            nc.gpsimd.collective_compute(
                "AllGather", ALU.bypass,
                replica_groups=[list(range(NCORES))],
                ins=[warm_in.opt()], outs=[warm_out.opt()],
            )

            ident = persist.tile([P, P], bf16, tag="ident")
            make_identity(nc, ident[:])
            ones1 = persist.tile([1, P], bf16, tag="ones1")
            nc.vector.memset(ones1[:], 1.0)
            ones8 = persist.tile([NCORES, 1], f32, tag="ones8")
            nc.vector.memset(ones8[:], 1.0)

            # --- embedding gather + relu + transpose ---
            idx_sb = persist.tile([B, 1], i32, tag="idx")
            nc.sync.dma_start(idx_sb[:], idx_d[:])
            x_bf = persist.tile([B, E], bf16, tag="x_bf")
            nc.gpsimd.indirect_dma_start(
                out=x_bf[:], out_offset=None,
                in_=emb_d[:],
                in_offset=bass.IndirectOffsetOnAxis(ap=idx_sb[:, :1], axis=0),
            )
            nc.vector.tensor_scalar_max(x_bf[:], x_bf[:], 0.0)

            xT = persist.tile([P, KC, B], bf16, tag="xT")
            for k in range(KC):
                pt = ps_tr.tile([P, P], bf16, tag="tr")
                nc.tensor.transpose(pt[:], x_bf[:, k * P:(k + 1) * P], ident[:])
                nc.vector.tensor_copy(xT[:, k, :], pt[:])

            # --- biases into SBUF (matmul rhs must be SBUF) ---
            gb_sb = persist.tile([1, 4, 3 * H], bf16, tag="gb_sb")
            nc.sync.dma_start(gb_sb[:], gb_d[:].rearrange("m o f -> o m f"))
            bo_sb = persist.tile([1, VS], bf16, tag="bo_sb")
            nc.sync.dma_start(bo_sb[:], bo_d[:])

            # --- hidden inputs ---
            hprev = [persist.tile([B, H], f32, tag=f"hprev{l}", name=f"hprev{l}") for l in range(2)]
            for l in range(2):
                nc.sync.dma_start(hprev[l][:], hid_d[l])
            hT_in = persist.tile([P, 2, KC, B], bf16, tag="hT_in")
            nc.sync.dma_start(
                hT_in[:], hidT_d[:].rearrange("l (o p) f -> p l o f", p=P))

            h_sb = [persist.tile([B, H], f32, tag=f"h_sb{l}", name=f"h_sb{l}") for l in range(2)]
            h0T = persist.tile([P, KC, B], bf16, tag="h0T")
            h1T = persist.tile([P, KC, B], bf16, tag="h1T")

            GCH = 512              # gate-column chunk
            NHC = H // GCH         # 2 H-chunks per layer

            for layer in range(2):
                inp_T = xT if layer == 0 else h0T
                w_ih = wT_d[2 * layer]        # [E, 3H]
                w_hh = wT_d[2 * layer + 1]
                b_ih = gb_sb[:, 2 * layer]    # [1, 3H]
                b_hh = gb_sb[:, 2 * layer + 1]

                for hc in range(NHC):
                    cols = {g: slice(g * H + hc * GCH, g * H + (hc + 1) * GCH)
                            for g in range(3)}  # 0=r, 1=z, 2=n
                    hT_l = hT_in[:, layer]

                    def _stream_w(wmat, g):
                        wt = wst.tile([P, KC, GCH], bf16, tag="wstream",
                                      name="wt")
                        nc.sync.dma_start(
                            wt[:],
                            wmat[:, cols[g]].rearrange("(o p) f -> p o f",
                                                       p=P))
                        return wt

                    # r and z gates: accumulate x@w_ih + b_ih + h@w_hh + b_hh
                    # into a single PSUM group
                    prz = []
                    for g in range(2):
                        ps = ps_g.tile([P, GCH], f32, tag=f"prz{g}",
                                       name=f"prz{g}")
                        prz.append(ps)
                        nc.tensor.matmul(ps[:], ones1[:], b_ih[:, cols[g]],
                                         start=True, stop=False)
                        wt = _stream_w(w_ih, g)
                        for k in range(KC):
                            nc.tensor.matmul(ps[:], inp_T[:, k, :], wt[:, k, :],
                                             start=False, stop=False)
                        nc.tensor.matmul(ps[:], ones1[:], b_hh[:, cols[g]],
                                         start=False, stop=False)
                        wt = _stream_w(w_hh, g)
                        for k in range(KC):
                            nc.tensor.matmul(ps[:], hT_l[:, k, :], wt[:, k, :],
                                             start=False, stop=(k == KC - 1))
                    # n gate: keep the two halves separate
                    pin = ps_g.tile([P, GCH], f32, tag="pin", name="pin")
                    nc.tensor.matmul(pin[:], ones1[:], b_ih[:, cols[2]],
                                     start=True, stop=False)
                    wt = _stream_w(w_ih, 2)
                    for k in range(KC):
                        nc.tensor.matmul(pin[:], inp_T[:, k, :], wt[:, k, :],
                                         start=False, stop=(k == KC - 1))
                    phn = ps_g.tile([P, GCH], f32, tag="phn", name="phn")
                    nc.tensor.matmul(phn[:], ones1[:], b_hh[:, cols[2]],
                                     start=True, stop=False)
                    wt = _stream_w(w_hh, 2)
                    for k in range(KC):
                        nc.tensor.matmul(phn[:], hT_l[:, k, :], wt[:, k, :],
                                         start=False, stop=(k == KC - 1))

                    hs = slice(hc * GCH, (hc + 1) * GCH)
                    r = tmp.tile([B, GCH], f32, tag="r")
                    nc.scalar.activation(r[:], prz[0][:], AF.Sigmoid)
                    z = tmp.tile([B, GCH], f32, tag="z")
                    nc.scalar.activation(z[:], prz[1][:], AF.Sigmoid)
                    n = tmp.tile([B, GCH], f32, tag="n")
                    nc.vector.tensor_tensor(n[:], r[:], phn[:], op=ALU.mult)
                    nc.vector.tensor_tensor(n[:], n[:], pin[:], op=ALU.add)
                    nc.scalar.activation(n[:], n[:], AF.Tanh)
                    # h' = n + z*(h - n)
                    d = tmp.tile([B, GCH], f32, tag="d")
                    nc.vector.tensor_tensor(d[:], hprev[layer][:, hs], n[:],
                                            op=ALU.subtract)
                    nc.vector.tensor_tensor(d[:], z[:], d[:], op=ALU.mult)
                    nc.vector.tensor_tensor(h_sb[layer][:, hs], d[:], n[:],
                                            op=ALU.add)

                # write hidden output + transposed bf16 copy for next matmul
                nc.sync.dma_start(hout_d[layer], h_sb[layer][:])
                hbf = tmp.tile([B, H], bf16, tag="hbf")
                nc.vector.tensor_copy(hbf[:], h_sb[layer][:])
                tgt = h0T if layer == 0 else h1T
                for k in range(KC):
                    pt = ps_tr.tile([P, P], bf16, tag="tr")
                    nc.tensor.transpose(pt[:], hbf[:, k * P:(k + 1) * P],
                                        ident[:])
                    nc.vector.tensor_copy(tgt[:, k, :], pt[:])

            # --- output projection: logits shard + exp-sums ---
            logits = persist.tile([B, VS], f32, tag="logits")
            sacc = persist.tile([B, NCORES], f32, tag="sacc")
            NVC = VS // GCH  # 8 chunks
            for c in range(NVC):
                vs = slice(c * GCH, (c + 1) * GCH)
                ps = ps_g.tile([P, GCH], f32, tag=["prz0", "prz1", "pin"][c % 3],
                               name="mmps")
                wt = wst.tile([P, KC, GCH], bf16, tag="wstream")
                nc.sync.dma_start(
                    wt[:], woT_d[:, vs].rearrange("(o p) f -> p o f", p=P))
                nc.tensor.matmul(ps[:], ones1[:], bo_sb[:, vs],
                                 start=True, stop=False)
                for k in range(KC):
                    nc.tensor.matmul(ps[:], h1T[:, k, :], wt[:, k, :],
                                     start=False, stop=(k == KC - 1))
                nc.vector.tensor_copy(logits[:, vs], ps[:])
                et = tmp.tile([B, GCH], f32, tag="exp")
                nc.scalar.activation(et[:], ps[:], AF.Exp,
                                     accum_out=sacc[:, c:c + 1])

            s_loc = persist.tile([B, 1], f32, tag="s_loc")
            nc.vector.reduce_sum(s_loc[:], sacc[:], axis=mybir.AxisListType.X)

            # --- exchange per-core sums, compute lse, normalize ---
            ag_in = dram.tile([1, B], f32, tag="ag_in")
            ag_out = dram.tile([NCORES, B], f32, tag="ag_out")
            nc.sync.dma_start(ag_in[:].rearrange("o p -> p o"), s_loc[:])
            nc.gpsimd.collective_compute(
                "AllGather", ALU.bypass,
                replica_groups=[list(range(NCORES))],
                ins=[ag_in.opt()], outs=[ag_out.opt()],
            )
            s_all = persist.tile([NCORES, B], f32, tag="s_all")
            nc.sync.dma_start(s_all[:], ag_out[:])
            ps_s = ps_g.tile([P, 1], f32, tag="phn")
            nc.tensor.matmul(ps_s[:], s_all[:], ones8[:], start=True, stop=True)
            lse = persist.tile([B, 1], f32, tag="lse")
            nc.scalar.activation(lse[:], ps_s[:], AF.Ln)

            for c in range(NVC):
                vs = slice(c * GCH, (c + 1) * GCH)
                ot = tmp.tile([B, GCH], f32, tag="norm")
                nc.vector.tensor_scalar(ot[:], logits[:, vs], lse[:, :1], None,
                                        op0=ALU.subtract)
                nc.sync.dma_start(out_d[:, vs], ot[:])

    nc.compile()
    return nc


def _prep_inputs(input_vector, hidden, emb, w_ih0, w_hh0, b_ih0, b_hh0,
                 w_ih1, w_hh1, b_ih1, b_hh1, w_out, b_out):
    bf = ml_dtypes.bfloat16
    idx = np.asarray(input_vector).astype(np.int32).reshape(B, 1)
    hid = np.ascontiguousarray(np.asarray(hidden, dtype=np.float32))
    hidT = np.ascontiguousarray(hid.transpose(0, 2, 1)).astype(bf)
    emb_bf = np.asarray(emb, dtype=np.float32).astype(bf)
    wT = np.stack([
        np.ascontiguousarray(np.asarray(w, dtype=np.float32).T).astype(bf)
        for w in (w_ih0, w_hh0, w_ih1, w_hh1)
    ])  # [4, E, 3H]
    gb = np.stack([
        np.asarray(b, dtype=np.float32).astype(bf).reshape(1, 3 * H)
        for b in (b_ih0, b_hh0, b_ih1, b_hh1)
    ])
    # padded, transposed, bf16 output projection
    woT_full = np.zeros((H, VPAD), dtype=bf)
    woT_full[:, :V] = np.ascontiguousarray(
        np.asarray(w_out, dtype=np.float32).T).astype(bf)
    bo_full = np.full((1, VPAD), NEG_BIG, dtype=bf)
    bo_full[0, :V] = np.asarray(b_out, dtype=np.float32).astype(bf)

    in_maps = []
    for c in range(NCORES):
        vs = slice(c * VS, (c + 1) * VS)
        in_maps.append({
            "emb": emb_bf,
            "idx": idx,
            "hid": hid,
            "hidT": hidT,
            "wT": wT,
            "gb": gb,
            "woT": np.ascontiguousarray(woT_full[:, vs]),
            "bo": np.ascontiguousarray(bo_full[:, vs]),
        })
    return in_maps


def kernel(**inputs):
    if "nc" not in _CACHE:
        _CACHE["nc"] = _build()
    nc = _CACHE["nc"]
    in_maps = _prep_inputs(**inputs)
    res = run_bass_kernel_spmd(nc, in_maps, list(range(NCORES)))
    out = np.concatenate(
        [res.results[c]["out"] for c in range(NCORES)], axis=1)[:, :V]
    hidden_out = res.results[0]["hout"]
    return np.asarray(out, dtype=np.float32), np.asarray(hidden_out,
                                                         dtype=np.float32)


# revision 18
# speedup vs baseline: 1.0352x; 1.0352x over previous
"""Trainium2 Bass kernel for nn_DecoderRNN (2-layer GRU decoder step + log_softmax).

Model (per reference):
    x  = relu(emb[input_vector])                    [B, E]
    h0 = gru_cell(x,  hidden[0], w_ih0, w_hh0, b_ih0, b_hh0)
    h1 = gru_cell(h0, hidden[1], w_ih1, w_hh1, b_ih1, b_hh1)
    out = log_softmax(h1 @ w_out.T + b_out)         [B, V]
    returns (out, stack([h0, h1]))

Sharding (8 NeuronCores):
  - The GRU (B=128, H=1024) is replicated on every core: at B=128 the matmul
    time is set by the moving (gate) dimension, so batch/tensor splits of the
    GRU only add collectives, which cost ~12-60us each on this runtime.
  - The output projection (V=32001) is column-parallel: each core owns a
    4096-wide vocab shard of w_out.T/b_out, computes its logits shard and the
    local sum(exp(logits)); one AllGather exchanges the per-core sums so every
    core can normalize its shard (log_softmax) on device.
  - A dependency-free tiny AllGather is issued at kernel start to absorb the
    per-execution collective-firmware warmup (~60us) under the weight DMA.

All matmuls run in bf16 (weights pre-cast on host) with fp32 PSUM
accumulation; gate math, softmax math, and outputs are fp32.
"""

import numpy as np
import ml_dtypes

import concourse.bass as bass
import concourse.mybir as mybir
import concourse.tile as tile
from concourse import bacc
from concourse.bass_utils import run_bass_kernel_spmd
from concourse.masks import make_identity

# Problem constants (hardcoded per harness contract)
B = 128
E = 1024
H = 1024
V = 32001
NCORES = 8
VS = 4096            # vocab shard per core (8 * 4096 = 32768 >= 32001, padded)
VPAD = NCORES * VS
P = 128
KC = E // P          # 8 contraction chunks of 128
NEG_BIG = -1e30      # bias value for padded vocab entries -> exp() == 0

f32 = mybir.dt.float32
bf16 = mybir.dt.bfloat16
i32 = mybir.dt.int32
AF = mybir.ActivationFunctionType
ALU = mybir.AluOpType

_CACHE = {}


def _build():
    nc = bacc.Bacc("TRN2", target_bir_lowering=False, debug=False,
                   num_devices=NCORES)

    # ---- I/O ----
    GCH = 512              # gate/vocab column chunk
    NHC = H // GCH         # H-chunks per layer
    NVC = VS // GCH        # vocab chunks

    # weights pre-packed on host into the streamed SBUF tile layout:
    # wS[m, g, hc] is [P, KC*GCH] with row p = [o=0: f0..f511, o=1: ...]
    emb_d = nc.dram_tensor("emb", [V, E], bf16, kind="ExternalInput").ap()
    idx_d = nc.dram_tensor("idx", [B, 1], i32, kind="ExternalInput").ap()
    hid_d = nc.dram_tensor("hid", [2, B, H], f32, kind="ExternalInput").ap()
    hS_d = nc.dram_tensor("hS", [P, 2 * KC * B], bf16,
                          kind="ExternalInput").ap()
    wS_d = nc.dram_tensor("wS", [4, 3, NHC, P, KC * GCH], bf16,
                          kind="ExternalInput").ap()
    gb_d = nc.dram_tensor("gb", [4, 1, 3 * H], bf16, kind="ExternalInput").ap()
    woS_d = nc.dram_tensor("woS", [NVC, P, KC * GCH], bf16,
                           kind="ExternalInput").ap()
    bo_d = nc.dram_tensor("bo", [1, VS], bf16, kind="ExternalInput").ap()

    out_d = nc.dram_tensor("out", [B, VS], f32, kind="ExternalOutput").ap()
    hout_d = nc.dram_tensor("hout", [2, B, H], f32, kind="ExternalOutput").ap()

    with tile.TileContext(nc) as tc:
        with (
            tc.tile_pool(name="persist", bufs=1) as persist,
            tc.tile_pool(name="wst", bufs=3) as wst,
            tc.tile_pool(name="tmp", bufs=3) as tmp,
            tc.tile_pool(name="dram", bufs=1, space="DRAM") as dram,
            tc.tile_pool(name="ps_tr", bufs=2, space="PSUM") as ps_tr,
            tc.tile_pool(name="ps_g", bufs=1, space="PSUM") as ps_g,
        ):
            # --- dummy collective: starts the ncfw warmup clock at t~0 ---
            warm_in = dram.tile([1, 16], f32, tag="warm_in")
            warm_out = dram.tile([NCORES, 16], f32, tag="warm_out")
            nc.gpsimd.collective_compute(
                "AllGather", ALU.bypass,
                replica_groups=[list(range(NCORES))],
                ins=[warm_in.opt()], outs=[warm_out.opt()],
            )

            ident = persist.tile([P, P], bf16, tag="ident")
            make_identity(nc, ident[:])
            ones1 = persist.tile([1, P], bf16, tag="ones1")
            nc.vector.memset(ones1[:], 1.0)
            ones8 = persist.tile([NCORES, 1], f32, tag="ones8")
            nc.vector.memset(ones8[:], 1.0)

            # --- embedding gather + relu + transpose ---
            idx_sb = persist.tile([B, 1], i32, tag="idx")
            nc.sync.dma_start(idx_sb[:], idx_d[:])
            x_bf = persist.tile([B, E], bf16, tag="x_bf")
            nc.gpsimd.indirect_dma_start(
                out=x_bf[:], out_offset=None,
                in_=emb_d[:],
                in_offset=bass.IndirectOffsetOnAxis(ap=idx_sb[:, :1], axis=0),
            )
            nc.vector.tensor_scalar_max(x_bf[:], x_bf[:], 0.0)

            xT = persist.tile([P, KC, B], bf16, tag="xT")
            for k in range(KC):
                pt = ps_tr.tile([P, P], bf16, tag="tr")
                nc.tensor.transpose(pt[:], x_bf[:, k * P:(k + 1) * P], ident[:])
                nc.vector.tensor_copy(xT[:, k, :], pt[:])

            # --- biases into SBUF (matmul rhs must be SBUF) ---
            gb_sb = persist.tile([1, 4, 3 * H], bf16, tag="gb_sb")
            nc.sync.dma_start(gb_sb[:], gb_d[:].rearrange("m o f -> o m f"))
            bo_sb = persist.tile([1, VS], bf16, tag="bo_sb")
            nc.sync.dma_start(bo_sb[:], bo_d[:])

            # --- hidden inputs ---
            hprev = [persist.tile([B, H], f32, tag=f"hprev{l}", name=f"hprev{l}") for l in range(2)]
            for l in range(2):
                nc.sync.dma_start(hprev[l][:], hid_d[l])
            hT_in = persist.tile([P, 2, KC, B], bf16, tag="hT_in")
            nc.sync.dma_start(
                hT_in[:], hS_d[:].rearrange("p (l o f) -> p l o f", o=KC, f=B))

            h_sb = [persist.tile([B, H], f32, tag=f"h_sb{l}", name=f"h_sb{l}") for l in range(2)]
            h0T = persist.tile([P, KC, B], bf16, tag="h0T")
            h1T = persist.tile([P, KC, B], bf16, tag="h1T")

            for layer in range(2):
                inp_T = xT if layer == 0 else h0T
                w_ih = wS_d[2 * layer]        # [3, NHC, P, KC*GCH]
                w_hh = wS_d[2 * layer + 1]
                b_ih = gb_sb[:, 2 * layer]    # [1, 3H]
                b_hh = gb_sb[:, 2 * layer + 1]

                for hc in range(NHC):
                    cols = {g: slice(g * H + hc * GCH, g * H + (hc + 1) * GCH)
                            for g in range(3)}  # 0=r, 1=z, 2=n
                    hT_l = hT_in[:, layer]

                    def _stream_w(wmat, g):
                        wt = wst.tile([P, KC, GCH], bf16, tag="wstream",
                                      name="wt")
                        nc.sync.dma_start(
                            wt[:],
                            wmat[g, hc].rearrange("p (o f) -> p o f", f=GCH))
                        return wt

                    # r and z gates: accumulate x@w_ih + b_ih + h@w_hh + b_hh
                    # into a single PSUM group
                    prz = []
                    for g in range(2):
                        ps = ps_g.tile([P, GCH], f32, tag=f"prz{g}",
                                       name=f"prz{g}")
                        prz.append(ps)
                        nc.tensor.matmul(ps[:], ones1[:], b_ih[:, cols[g]],
                                         start=True, stop=False)
                        wt = _stream_w(w_ih, g)
                        for k in range(KC):
                            nc.tensor.matmul(ps[:], inp_T[:, k, :], wt[:, k, :],
                                             start=False, stop=False)
                        nc.tensor.matmul(ps[:], ones1[:], b_hh[:, cols[g]],
                                         start=False, stop=False)
                        wt = _stream_w(w_hh, g)
                        for k in range(KC):
                            nc.tensor.matmul(ps[:], hT_l[:, k, :], wt[:, k, :],
                                             start=False, stop=(k == KC - 1))
                    # n gate: keep the two halves separate
                    pin = ps_g.tile([P, GCH], f32, tag="pin", name="pin")
                    nc.tensor.matmul(pin[:], ones1[:], b_ih[:, cols[2]],
                                     start=True, stop=False)
                    wt = _stream_w(w_ih, 2)
                    for k in range(KC):
                        nc.tensor.matmul(pin[:], inp_T[:, k, :], wt[:, k, :],
                                         start=False, stop=(k == KC - 1))
                    phn = ps_g.tile([P, GCH], f32, tag="phn", name="phn")
                    nc.tensor.matmul(phn[:], ones1[:], b_hh[:, cols[2]],
                                     start=True, stop=False)
                    wt = _stream_w(w_hh, 2)
                    for k in range(KC):
                        nc.tensor.matmul(phn[:], hT_l[:, k, :], wt[:, k, :],
                                         start=False, stop=(k == KC - 1))

                    hs = slice(hc * GCH, (hc + 1) * GCH)
                    r = tmp.tile([B, GCH], f32, tag="r")
                    nc.scalar.activation(r[:], prz[0][:], AF.Sigmoid)
                    z = tmp.tile([B, GCH], f32, tag="z")
                    nc.scalar.activation(z[:], prz[1][:], AF.Sigmoid)
                    n = tmp.tile([B, GCH], f32, tag="n")
                    nc.vector.tensor_tensor(n[:], r[:], phn[:], op=ALU.mult)
                    nc.vector.tensor_tensor(n[:], n[:], pin[:], op=ALU.add)
                    nc.scalar.activation(n[:], n[:], AF.Tanh)
                    # h' = n + z*(h - n)
                    d = tmp.tile([B, GCH], f32, tag="d")
                    nc.vector.tensor_tensor(d[:], hprev[layer][:, hs], n[:],
                                            op=ALU.subtract)
                    nc.vector.tensor_tensor(d[:], z[:], d[:], op=ALU.mult)
                    nc.vector.tensor_tensor(h_sb[layer][:, hs], d[:], n[:],
                                            op=ALU.add)

                # write hidden output + transposed bf16 copy for next matmul
                nc.sync.dma_start(hout_d[layer], h_sb[layer][:])
                hbf = tmp.tile([B, H], bf16, tag="hbf")
                nc.vector.tensor_copy(hbf[:], h_sb[layer][:])
                tgt = h0T if layer == 0 else h1T
                for k in range(KC):
                    pt = ps_tr.tile([P, P], bf16, tag="tr")
                    nc.tensor.transpose(pt[:], hbf[:, k * P:(k + 1) * P],
                                        ident[:])
                    nc.vector.tensor_copy(tgt[:, k, :], pt[:])

            # --- output projection: logits shard + exp-sums ---
            logits = persist.tile([B, VS], f32, tag="logits")
            sacc = persist.tile([B, NCORES], f32, tag="sacc")
            NVC = VS // GCH  # 8 chunks
            for c in range(NVC):
                vs = slice(c * GCH, (c + 1) * GCH)
                ps = ps_g.tile([P, GCH], f32, tag=["prz0", "prz1", "pin"][c % 3],
                               name="mmps")
                wt = wst.tile([P, KC, GCH], bf16, tag="wstream")
                nc.sync.dma_start(
                    wt[:], woS_d[c].rearrange("p (o f) -> p o f", f=GCH))
                nc.tensor.matmul(ps[:], ones1[:], bo_sb[:, vs],
                                 start=True, stop=False)
                for k in range(KC):
                    nc.tensor.matmul(ps[:], h1T[:, k, :], wt[:, k, :],
                                     start=False, stop=(k == KC - 1))
                nc.vector.tensor_copy(logits[:, vs], ps[:])
                et = tmp.tile([B, GCH], f32, tag="exp")
                nc.scalar.activation(et[:], ps[:], AF.Exp,
                                     accum_out=sacc[:, c:c + 1])

            s_loc = persist.tile([B, 1], f32, tag="s_loc")
            nc.vector.reduce_sum(s_loc[:], sacc[:], axis=mybir.AxisListType.X)

            # --- exchange per-core sums, compute lse, normalize ---
            ag_in = dram.tile([1, B], f32, tag="ag_in")
            ag_out = dram.tile([NCORES, B], f32, tag="ag_out")
            nc.sync.dma_start(ag_in[:].rearrange("o p -> p o"), s_loc[:])
            nc.gpsimd.collective_compute(
                "AllGather", ALU.bypass,
                replica_groups=[list(range(NCORES))],
                ins=[ag_in.opt()], outs=[ag_out.opt()],
            )
            s_all = persist.tile([NCORES, B], f32, tag="s_all")
            nc.sync.dma_start(s_all[:], ag_out[:])
            ps_s = ps_g.tile([P, 1], f32, tag="phn")
            nc.tensor.matmul(ps_s[:], s_all[:], ones8[:], start=True, stop=True)
            lse = persist.tile([B, 1], f32, tag="lse")
            nc.scalar.activation(lse[:], ps_s[:], AF.Ln)

            for c in range(NVC):
                vs = slice(c * GCH, (c + 1) * GCH)
                ot = tmp.tile([B, GCH], f32, tag="norm")
                nc.vector.tensor_scalar(ot[:], logits[:, vs], lse[:, :1], None,
                                        op0=ALU.subtract)
                nc.sync.dma_start(out_d[:, vs], ot[:])

    nc.compile()
    return nc


def _pack_stream(T, gch=512):
    """[E, F] -> [F//gch, P, KC*gch] in the streamed SBUF tile layout."""
    E_, F = T.shape
    nch = F // gch
    # chunk j, partition p, flat (o, f): T[o*P + p, j*gch + f]
    return np.ascontiguousarray(
        T.reshape(E_ // P, P, nch, gch)      # [o, p, j, f]
        .transpose(2, 1, 0, 3)               # [j, p, o, f]
        .reshape(nch, P, (E_ // P) * gch))


def _prep_inputs(input_vector, hidden, emb, w_ih0, w_hh0, b_ih0, b_hh0,
                 w_ih1, w_hh1, b_ih1, b_hh1, w_out, b_out):
    bf = ml_dtypes.bfloat16
    NHC = H // 512
    NVC = VS // 512
    idx = np.asarray(input_vector).astype(np.int32).reshape(B, 1)
    hid = np.ascontiguousarray(np.asarray(hidden, dtype=np.float32))
    # hS[p, (l, o, f)] = hidden[l][f, o*P + p]
    hS = np.ascontiguousarray(
        hid.transpose(0, 2, 1).astype(bf)    # [2, H, B]
        .reshape(2, KC, P, B)                # [l, o, p, f]
        .transpose(2, 0, 1, 3)               # [p, l, o, f]
        .reshape(P, 2 * KC * B))
    emb_bf = np.asarray(emb, dtype=np.float32).astype(bf)
    # wS[m, g, hc] = packed [P, KC*512] tile of w_m.T gate-column chunk
    wS = np.stack([
        _pack_stream(np.asarray(w, dtype=np.float32).T.astype(bf))
        .reshape(3, NHC, P, KC * 512)
        for w in (w_ih0, w_hh0, w_ih1, w_hh1)
    ])
    gb = np.stack([
        np.asarray(b, dtype=np.float32).astype(bf).reshape(1, 3 * H)
        for b in (b_ih0, b_hh0, b_ih1, b_hh1)
    ])
    # padded, transposed, bf16 output projection
    woT_full = np.zeros((H, VPAD), dtype=bf)
    woT_full[:, :V] = np.ascontiguousarray(
        np.asarray(w_out, dtype=np.float32).T).astype(bf)
    bo_full = np.full((1, VPAD), NEG_BIG, dtype=bf)
    bo_full[0, :V] = np.asarray(b_out, dtype=np.float32).astype(bf)

    in_maps = []
    for c in range(NCORES):
        vs = slice(c * VS, (c + 1) * VS)
        in_maps.append({
            "emb": emb_bf,
            "idx": idx,
            "hid": hid,
            "hS": hS,
            "wS": wS,
            "gb": gb,
            "woS": _pack_stream(woT_full[:, vs]),
            "bo": np.ascontiguousarray(bo_full[:, vs]),
        })
    return in_maps


def kernel(**inputs):
    if "nc" not in _CACHE:
        _CACHE["nc"] = _build()
    nc = _CACHE["nc"]
    in_maps = _prep_inputs(**inputs)
    res = run_bass_kernel_spmd(nc, in_maps, list(range(NCORES)))
    out = np.concatenate(
        [res.results[c]["out"] for c in range(NCORES)], axis=1)[:, :V]
    hidden_out = res.results[0]["hout"]
    return np.asarray(out, dtype=np.float32), np.asarray(hidden_out,
                                                         dtype=np.float32)


# revision 27
# speedup vs baseline: 1.1575x; 1.1181x over previous
"""Trainium2 Bass kernel for nn_DecoderRNN (2-layer GRU decoder step + log_softmax).

Model (per reference):
    x  = relu(emb[input_vector])                    [B, E]
    h0 = gru_cell(x,  hidden[0], w_ih0, w_hh0, b_ih0, b_hh0)
    h1 = gru_cell(h0, hidden[1], w_ih1, w_hh1, b_ih1, b_hh1)
    out = log_softmax(h1 @ w_out.T + b_out)         [B, V]
    returns (out, stack([h0, h1]))

Sharding (8 NeuronCores):
  - The GRU (B=128, H=1024) is replicated on every core: at B=128 the matmul
    time is set by the moving (gate) dimension, so batch/tensor splits of the
    GRU only add collectives, which cost ~12-60us each on this runtime.
  - The output projection (V=32001) is column-parallel: each core owns a
    4096-wide vocab shard of w_out.T/b_out, computes its logits shard and the
    local sum(exp(logits)); one AllGather exchanges the per-core sums so every
    core can normalize its shard (log_softmax) on device.
  - A dependency-free tiny AllGather is issued at kernel start to absorb the
    per-execution collective-firmware warmup (~60us) under the weight DMA.

All matmuls run in bf16 (weights pre-cast on host) with fp32 PSUM
accumulation; gate math, softmax math, and outputs are fp32.
"""

import numpy as np
import ml_dtypes

import concourse.bass as bass
import concourse.mybir as mybir
import concourse.tile as tile
from concourse import bacc
from concourse.bass_utils import run_bass_kernel_spmd
from concourse.masks import make_identity

# Problem constants (hardcoded per harness contract)
B = 128
E = 1024
H = 1024
V = 32001
NCORES = 8
VS = 4096            # vocab shard per core (8 * 4096 = 32768 >= 32001, padded)
VPAD = NCORES * VS
P = 128
KC = E // P          # 8 contraction chunks of 128
NEG_BIG = -1e30      # bias value for padded vocab entries -> exp() == 0

f32 = mybir.dt.float32
bf16 = mybir.dt.bfloat16
i32 = mybir.dt.int32
AF = mybir.ActivationFunctionType
ALU = mybir.AluOpType

_CACHE = {}


def _build():
    nc = bacc.Bacc("TRN2", target_bir_lowering=False, debug=False,
                   num_devices=NCORES)

    # ---- I/O ----
    GCH = 512              # gate/vocab column chunk
    NHC = H // GCH         # H-chunks per layer
    NVC = VS // GCH        # vocab chunks

    # weights pre-packed on host into the streamed SBUF tile layout:
    # wS[m, g, hc] is [P, KC*GCH] with row p = [o=0: f0..f511, o=1: ...]
    emb_d = nc.dram_tensor("emb", [V, E], bf16, kind="ExternalInput").ap()
    idx_d = nc.dram_tensor("idx", [B, 1], i32, kind="ExternalInput").ap()
    hid_d = nc.dram_tensor("hid", [2, B, H], f32, kind="ExternalInput").ap()
    hS_d = nc.dram_tensor("hS", [P, 2 * KC * B], bf16,
                          kind="ExternalInput").ap()
    wS_d = nc.dram_tensor("wS", [4, NHC, P, KC * 3 * GCH], bf16,
                          kind="ExternalInput").ap()
    gb_d = nc.dram_tensor("gb", [4, 1, 3 * H], bf16, kind="ExternalInput").ap()
    woS_d = nc.dram_tensor("woS", [NVC // 2, P, KC * 2 * GCH], bf16,
                           kind="ExternalInput").ap()
    bo_d = nc.dram_tensor("bo", [1, VS], bf16, kind="ExternalInput").ap()

    out_d = nc.dram_tensor("out", [B, VS], f32, kind="ExternalOutput").ap()
    hout_d = nc.dram_tensor("hout", [2, B, H], f32, kind="ExternalOutput").ap()

    with tile.TileContext(nc) as tc:
        with (
            tc.tile_pool(name="persist", bufs=1) as persist,
            tc.tile_pool(name="wst", bufs=3) as wst,
            tc.tile_pool(name="tmp", bufs=2) as tmp,
            tc.tile_pool(name="dram", bufs=1, space="DRAM") as dram,
            tc.tile_pool(name="ps_tr", bufs=2, space="PSUM") as ps_tr,
            tc.tile_pool(name="ps_g", bufs=1, space="PSUM") as ps_g,
        ):
            # --- dummy collective: starts the ncfw warmup clock at t~0 ---
            warm_in = dram.tile([1, 16], f32, tag="warm_in")
            warm_out = dram.tile([NCORES, 16], f32, tag="warm_out")
            nc.gpsimd.collective_compute(
                "AllGather", ALU.bypass,
                replica_groups=[list(range(NCORES))],
                ins=[warm_in.opt()], outs=[warm_out.opt()],
            )

            ident = persist.tile([P, P], bf16, tag="ident")
            make_identity(nc, ident[:])
            ones1 = persist.tile([1, P], bf16, tag="ones1")
            nc.vector.memset(ones1[:], 1.0)
            ones8 = persist.tile([NCORES, 1], f32, tag="ones8")
            nc.vector.memset(ones8[:], 1.0)

            # --- embedding gather + relu + transpose ---
            idx_sb = persist.tile([B, 1], i32, tag="idx")
            nc.sync.dma_start(idx_sb[:], idx_d[:])
            x_bf = persist.tile([B, E], bf16, tag="x_bf")
            nc.gpsimd.indirect_dma_start(
                out=x_bf[:], out_offset=None,
                in_=emb_d[:],
                in_offset=bass.IndirectOffsetOnAxis(ap=idx_sb[:, :1], axis=0),
            )
            nc.vector.tensor_scalar_max(x_bf[:], x_bf[:], 0.0)

            xT = persist.tile([P, KC, B], bf16, tag="xT")
            for k in range(KC):
                pt = ps_tr.tile([P, P], bf16, tag="tr")
                nc.tensor.transpose(pt[:], x_bf[:, k * P:(k + 1) * P], ident[:])
                nc.vector.tensor_copy(xT[:, k, :], pt[:])

            # --- biases into SBUF (matmul rhs must be SBUF) ---
            gb_sb = persist.tile([1, 4, 3 * H], bf16, tag="gb_sb")
            nc.sync.dma_start(gb_sb[:], gb_d[:].rearrange("m o f -> o m f"))
            bo_sb = persist.tile([1, VS], bf16, tag="bo_sb")
            nc.sync.dma_start(bo_sb[:], bo_d[:])

            # --- hidden inputs ---
            hprev = [persist.tile([B, H], f32, tag=f"hprev{l}", name=f"hprev{l}") for l in range(2)]
            for l in range(2):
                nc.sync.dma_start(hprev[l][:], hid_d[l])
            hT_in = persist.tile([P, 2, KC, B], bf16, tag="hT_in")
            nc.sync.dma_start(
                hT_in[:], hS_d[:].rearrange("p (l o f) -> p l o f", o=KC, f=B))

            h_sb = [persist.tile([B, H], f32, tag=f"h_sb{l}", name=f"h_sb{l}") for l in range(2)]
            h0T = persist.tile([P, KC, B], bf16, tag="h0T")
            h1T = persist.tile([P, KC, B], bf16, tag="h1T")

            for layer in range(2):
                inp_T = xT if layer == 0 else h0T
                w_ih = wS_d[2 * layer]        # [3, NHC, P, KC*GCH]
                w_hh = wS_d[2 * layer + 1]
                b_ih = gb_sb[:, 2 * layer]    # [1, 3H]
                b_hh = gb_sb[:, 2 * layer + 1]

                for hc in range(NHC):
                    cols = {g: slice(g * H + hc * GCH, g * H + (hc + 1) * GCH)
                            for g in range(3)}  # 0=r, 1=z, 2=n
                    hT_l = hT_in[:, layer]

                    # one 3MB DMA per (matrix, h-chunk): all 3 gate chunks
                    wti = wst.tile([P, KC, 3, GCH], bf16, tag="wstream",
                                   name="wti")
                    nc.sync.dma_start(
                        wti[:], w_ih[hc].rearrange("p (o g f) -> p o g f",
                                                   g=3, f=GCH))
                    wth = wst.tile([P, KC, 3, GCH], bf16, tag="wstream",
                                   name="wth")
                    nc.sync.dma_start(
                        wth[:], w_hh[hc].rearrange("p (o g f) -> p o g f",
                                                   g=3, f=GCH))

                    # r and z gates: accumulate x@w_ih + b_ih + h@w_hh + b_hh
                    # into a single PSUM group
                    prz = []
                    for g in range(2):
                        ps = ps_g.tile([P, GCH], f32, tag=f"prz{g}",
                                       name=f"prz{g}")
                        prz.append(ps)
                        nc.tensor.matmul(ps[:], ones1[:], b_ih[:, cols[g]],
                                         start=True, stop=False)
                        for k in range(KC):
                            nc.tensor.matmul(ps[:], inp_T[:, k, :],
                                             wti[:, k, g, :],
                                             start=False, stop=False)
                        nc.tensor.matmul(ps[:], ones1[:], b_hh[:, cols[g]],
                                         start=False, stop=False)
                        for k in range(KC):
                            nc.tensor.matmul(ps[:], hT_l[:, k, :],
                                             wth[:, k, g, :],
                                             start=False, stop=(k == KC - 1))
                    # n gate: keep the two halves separate
                    pin = ps_g.tile([P, GCH], f32, tag="pin", name="pin")
                    nc.tensor.matmul(pin[:], ones1[:], b_ih[:, cols[2]],
                                     start=True, stop=False)
                    for k in range(KC):
                        nc.tensor.matmul(pin[:], inp_T[:, k, :],
                                         wti[:, k, 2, :],
                                         start=False, stop=(k == KC - 1))
                    phn = ps_g.tile([P, GCH], f32, tag="phn", name="phn")
                    nc.tensor.matmul(phn[:], ones1[:], b_hh[:, cols[2]],
                                     start=True, stop=False)
                    for k in range(KC):
                        nc.tensor.matmul(phn[:], hT_l[:, k, :],
                                         wth[:, k, 2, :],
                                         start=False, stop=(k == KC - 1))

                    hs = slice(hc * GCH, (hc + 1) * GCH)
                    r = tmp.tile([B, GCH], f32, tag="r")
                    nc.scalar.activation(r[:], prz[0][:], AF.Sigmoid)
                    z = tmp.tile([B, GCH], f32, tag="z")
                    nc.scalar.activation(z[:], prz[1][:], AF.Sigmoid)
                    n = tmp.tile([B, GCH], f32, tag="n")
                    nc.vector.tensor_tensor(n[:], r[:], phn[:], op=ALU.mult)
                    nc.vector.tensor_tensor(n[:], n[:], pin[:], op=ALU.add)
                    nc.scalar.activation(n[:], n[:], AF.Tanh)
                    # h' = n + z*(h - n)
                    d = tmp.tile([B, GCH], f32, tag="d")
                    nc.vector.tensor_tensor(d[:], hprev[layer][:, hs], n[:],
                                            op=ALU.subtract)
                    nc.vector.tensor_tensor(d[:], z[:], d[:], op=ALU.mult)
                    nc.vector.tensor_tensor(h_sb[layer][:, hs], d[:], n[:],
                                            op=ALU.add)

                # write hidden output + transposed bf16 copy for next matmul
                nc.sync.dma_start(hout_d[layer], h_sb[layer][:])
                hbf = tmp.tile([B, H], bf16, tag="hbf")
                nc.vector.tensor_copy(hbf[:], h_sb[layer][:])
                tgt = h0T if layer == 0 else h1T
                for k in range(KC):
                    pt = ps_tr.tile([P, P], bf16, tag="tr")
                    nc.tensor.transpose(pt[:], hbf[:, k * P:(k + 1) * P],
                                        ident[:])
                    nc.vector.tensor_copy(tgt[:, k, :], pt[:])

            # --- output projection: logits shard + exp-sums ---
            logits = persist.tile([B, VS], f32, tag="logits")
            sacc = persist.tile([B, NCORES], f32, tag="sacc")
            for c2 in range(NVC // 2):
                wt = wst.tile([P, KC, 2, GCH], bf16, tag="wstream",
                              name="wt2")
                nc.sync.dma_start(
                    wt[:], woS_d[c2].rearrange("p (o j f) -> p o j f",
                                               j=2, f=GCH))
                for j in range(2):
                    c = 2 * c2 + j
                    vs = slice(c * GCH, (c + 1) * GCH)
                    ps = ps_g.tile([P, GCH], f32,
                                   tag=["prz0", "prz1", "pin"][c % 3],
                                   name="mmps")
                    nc.tensor.matmul(ps[:], ones1[:], bo_sb[:, vs],
                                     start=True, stop=False)
                    for k in range(KC):
                        nc.tensor.matmul(ps[:], h1T[:, k, :], wt[:, k, j, :],
                                         start=False, stop=(k == KC - 1))
                    nc.vector.tensor_copy(logits[:, vs], ps[:])
                    et = tmp.tile([B, GCH], f32, tag="exp")
                    nc.scalar.activation(et[:], ps[:], AF.Exp,
                                         accum_out=sacc[:, c:c + 1])

            s_loc = persist.tile([B, 1], f32, tag="s_loc")
            nc.vector.reduce_sum(s_loc[:], sacc[:], axis=mybir.AxisListType.X)

            # --- exchange per-core sums, compute lse, normalize ---
            ag_in = dram.tile([1, B], f32, tag="ag_in")
            ag_out = dram.tile([NCORES, B], f32, tag="ag_out")
            nc.sync.dma_start(ag_in[:].rearrange("o p -> p o"), s_loc[:])
            nc.gpsimd.collective_compute(
                "AllGather", ALU.bypass,
                replica_groups=[list(range(NCORES))],
                ins=[ag_in.opt()], outs=[ag_out.opt()],
            )
            s_all = persist.tile([NCORES, B], f32, tag="s_all")
            nc.sync.dma_start(s_all[:], ag_out[:])
            ps_s = ps_g.tile([P, 1], f32, tag="phn")
            nc.tensor.matmul(ps_s[:], s_all[:], ones8[:], start=True, stop=True)
            lse = persist.tile([B, 1], f32, tag="lse")
            nc.scalar.activation(lse[:], ps_s[:], AF.Ln)

            for c in range(NVC):
                vs = slice(c * GCH, (c + 1) * GCH)
                ot = tmp.tile([B, GCH], f32, tag="norm")
                nc.vector.tensor_scalar(ot[:], logits[:, vs], lse[:, :1], None,
                                        op0=ALU.subtract)
                nc.sync.dma_start(out_d[:, vs], ot[:])

    nc.compile()
    return nc


def _pack_stream(T, grp):
    """[E, F] -> [F//grp, P, KC*grp]: chunk j, partition p, flat (o, f) =
    T[o*P + p, j*grp + f]."""
    E_, F = T.shape
    nch = F // grp
    return np.ascontiguousarray(
        T.reshape(E_ // P, P, nch, grp)      # [o, p, j, f]
        .transpose(2, 1, 0, 3)               # [j, p, o, f]
        .reshape(nch, P, (E_ // P) * grp))


def _prep_inputs(input_vector, hidden, emb, w_ih0, w_hh0, b_ih0, b_hh0,
                 w_ih1, w_hh1, b_ih1, b_hh1, w_out, b_out):
    bf = ml_dtypes.bfloat16
    NHC = H // 512
    NVC = VS // 512
    idx = np.asarray(input_vector).astype(np.int32).reshape(B, 1)
    hid = np.ascontiguousarray(np.asarray(hidden, dtype=np.float32))
    # hS[p, (l, o, f)] = hidden[l][f, o*P + p]
    hS = np.ascontiguousarray(
        hid.transpose(0, 2, 1).astype(bf)    # [2, H, B]
        .reshape(2, KC, P, B)                # [l, o, p, f]
        .transpose(2, 0, 1, 3)               # [p, l, o, f]
        .reshape(P, 2 * KC * B))
    emb_bf = np.asarray(emb, dtype=np.float32).astype(bf)
    # wS[m, hc, p, (o, g, f)] = w_m.T[o*P + p, g*H + hc*512 + f]
    def _pack_gru(w):
        T = np.asarray(w, dtype=np.float32).T.astype(bf)     # [E, 3H]
        return np.ascontiguousarray(
            T.reshape(KC, P, 3, NHC, 512)    # [o, p, g, hc, f]
            .transpose(3, 1, 0, 2, 4)        # [hc, p, o, g, f]
            .reshape(NHC, P, KC * 3 * 512))
    wS = np.stack([_pack_gru(w) for w in (w_ih0, w_hh0, w_ih1, w_hh1)])
    gb = np.stack([
        np.asarray(b, dtype=np.float32).astype(bf).reshape(1, 3 * H)
        for b in (b_ih0, b_hh0, b_ih1, b_hh1)
    ])
    # padded, transposed, bf16 output projection
    woT_full = np.zeros((H, VPAD), dtype=bf)
    woT_full[:, :V] = np.ascontiguousarray(
        np.asarray(w_out, dtype=np.float32).T).astype(bf)
    bo_full = np.full((1, VPAD), NEG_BIG, dtype=bf)
    bo_full[0, :V] = np.asarray(b_out, dtype=np.float32).astype(bf)

    in_maps = []
    for c in range(NCORES):
        vs = slice(c * VS, (c + 1) * VS)
        in_maps.append({
            "emb": emb_bf,
            "idx": idx,
            "hid": hid,
            "hS": hS,
            "wS": wS,
            "gb": gb,
            "woS": _pack_stream(woT_full[:, vs], 2 * 512),
            "bo": np.ascontiguousarray(bo_full[:, vs]),
        })
    return in_maps


def kernel(**inputs):
    if "nc" not in _CACHE:
        _CACHE["nc"] = _build()
    nc = _CACHE["nc"]
    in_maps = _prep_inputs(**inputs)
    res = run_bass_kernel_spmd(nc, in_maps, list(range(NCORES)))
    out = np.concatenate(
        [res.results[c]["out"] for c in range(NCORES)], axis=1)[:, :V]
    hidden_out = res.results[0]["hout"]
    return np.asarray(out, dtype=np.float32), np.asarray(hidden_out,
                                                         dtype=np.float32)
